# revision 38
# baseline (speedup 1.0000x reference)
"""Multi-head causal attention (B=4, S=2048, D=1024, H=16) on 8 trn2 NeuronCores.

Sharding: core = (batch b, head-group g) with b = core//2, g = core%2.
Each core computes batch b, heads g*8..g*8+8 fully locally (no collectives):
  - host pre-transposes x[b] -> xT [1024, 2048] and slices Wq/Wk/Wv columns.
  - projections: QT, KT = [512, 2048] (head-dim on partitions), V = [2048, 520]
    (65 cols/head: 64 value dims + a ones column that makes the PV matmul
    emit softmax denominators for free).
  - scores are computed transposed, S^T[k, q] = (KT slice).T @ (QT slice),
    so softmax sums reduce over the PSUM partition dim via the ones column
    and no transposes are needed anywhere.
  - no max-subtraction in softmax: scores/8 ~ N(0,1), exp cannot overflow.
  - causal masking: fully-masked k-tiles are skipped; diagonal tiles exp and
    mask only the valid columns (strided two-head APs).
  - head pairs share the PE array: the two K=64 score matmuls go to disjoint
    row groups (base partitions 0/64) and run concurrently on hardware.
  - matmul dtypes: fp16 inputs for the projections (errors average over the
    1024-long contraction), float32r (tf32-like, 1 cyc/row) for scores,
    fp16 for exp(S) @ V (softmax weights tolerate fp16; values <= e^6).
  - emission is a software pipeline: exp cadence rules ScalarE, so score
    units are interleaved with ~1.25us pieces of projection/PV filler work.
"""
import sys

for _p in ("/opt/trn_rl_repo",):
    if _p not in sys.path:
        sys.path.insert(0, _p)

import numpy as np
import concourse.bacc as bacc
import concourse.mybir as mybir
from concourse.tile import TileContext
from concourse.bass_utils import run_bass_kernel_spmd

FP32 = mybir.dt.float32
F32R = mybir.dt.float32r
FP16 = mybir.dt.float16

B, S, D, H, HD = 4, 2048, 1024, 16, 64
NCORES = 8
HPC = 8          # heads per core
DG = HPC * HD    # 512 output cols per core
CT = 128         # contraction tile
NCT = D // CT    # 8
QC = 512         # q chunk (matmul N)
KT = 128         # k tile
SCALE = 1.0 / np.sqrt(HD)


def build_nc(seq=S):
    nqc = seq // QC          # q chunks
    nqt = seq // KT          # q tiles of 128
    nst = seq // KT          # seq tiles for V
    nmc = seq // QC          # m chunks in projections

    nc = bacc.Bacc()
    xT = nc.dram_tensor("xT", [D, seq], FP16, kind="ExternalInput")
    wq = nc.dram_tensor("wq", [D, DG], FP16, kind="ExternalInput")
    wk = nc.dram_tensor("wk", [D, DG], FP16, kind="ExternalInput")
    wv = nc.dram_tensor("wv", [D, DG], FP16, kind="ExternalInput")
    masks = nc.dram_tensor("masks", [4 * KT, 2 * QC], FP16, kind="ExternalInput")
    out = nc.dram_tensor("out", [seq, DG], FP32, kind="ExternalOutput")

    with TileContext(nc) as tc:
        with tc.tile_pool(name="big", bufs=1) as big, \
             tc.tile_pool(name="wp", bufs=50) as wp, \
             tc.tile_pool(name="wvp", bufs=8) as wvp, \
             tc.tile_pool(name="work", bufs=4) as work, \
             tc.tile_pool(name="pt", bufs=24) as ptp, \
             tc.tile_pool(name="outp", bufs=6) as outp, \
             tc.tile_pool(name="ps_proj", bufs=2, space="PSUM") as ps_proj, \
             tc.tile_pool(name="ps_s", bufs=2, space="PSUM") as ps_s, \
             tc.tile_pool(name="ps_c", bufs=2, space="PSUM") as ps_c:

            # ---- resident tiles ----
            xt_tiles = []
            for ct in range(NCT):
                t = big.tile([128, seq], FP16, tag=f"xt{ct}", name=f"xt{ct}")
                nc.sync.dma_start(out=t, in_=xT[ct * CT:(ct + 1) * CT, :])
                xt_tiles.append(t)
            qt_tiles = [big.tile([128, seq], F32R, tag=f"qt{dp}", name=f"qt{dp}") for dp in range(4)]
            kt_tiles = [big.tile([128, seq], F32R, tag=f"kt{dp}", name=f"kt{dp}") for dp in range(4)]
            v65 = [big.tile([128, HPC * 65], FP16, tag=f"v{st}", name=f"v{st}") for st in range(nst)]
            mask_t = big.tile([128, 4 * 2 * QC], FP16, tag="masks", name="mask_t")

            wt = {}       # (proj, dp, ct) -> w subtile

            def emit_w_dma(proj, dp):
                w_dram = wq if proj == 0 else wk
                for ct in range(NCT):
                    t = wp.tile([128, 128], FP16, tag="w", name="w")
                    nc.sync.dma_start(
                        out=t,
                        in_=w_dram[ct * CT:(ct + 1) * CT, dp * 128:(dp + 1) * 128])
                    wt[(proj, dp, ct)] = t

            wv_tiles = []

            def emit_wv_load():
                for ct in range(NCT):
                    t = wvp.tile([128, DG], FP16, tag="wv", name="wv")
                    nc.sync.dma_start(out=t, in_=wv[ct * CT:(ct + 1) * CT, :])
                    wv_tiles.append(t)

            def emit_score_unit(dp, qc, kt, pt_tiles):
                """Scores + exp + mask for one k-tile, both heads of the pair.
                The two heads' K=64 matmuls target disjoint PE row groups
                (base partitions 0/64), so hardware runs them concurrently."""
                spsum = ps_s.tile([128, 2 * QC], FP32, tag="s", name="s_ps")
                for hh in (0, 1):
                    nc.tensor.matmul(
                        spsum[:, hh * QC:(hh + 1) * QC],
                        lhsT=kt_tiles[dp][hh * 64:hh * 64 + 64,
                                          kt * KT:(kt + 1) * KT],
                        rhs=qt_tiles[dp][hh * 64:hh * 64 + 64,
                                         qc * QC:(qc + 1) * QC],
                        start=True, stop=True)
                pt = ptp.tile([128, 2 * QC], FP16, tag="pt", name="pt")
                o_idx = kt - 4 * qc
                o = max(o_idx, 0) * KT
                if o == 0:
                    nc.scalar.activation(
                        out=pt, in_=spsum,
                        func=mybir.ActivationFunctionType.Exp,
                        scale=SCALE)
                else:
                    # diagonal tile: columns < o are fully masked for every
                    # k row in this tile and never read downstream - skip
                    # them in exp via a strided two-head AP
                    nc.scalar.activation(
                        out=pt.rearrange("p (h q) -> p h q", q=QC)[:, :, o:],
                        in_=spsum.rearrange("p (h q) -> p h q", q=QC)[:, :, o:],
                        func=mybir.ActivationFunctionType.Exp,
                        scale=SCALE)
                if o_idx >= 0:   # triangular mask on the valid columns
                    m2 = mask_t[:, o_idx * 2 * QC:(o_idx + 1) * 2 * QC]
                    nc.vector.tensor_mul(
                        out=pt.rearrange("p (h q) -> p h q", q=QC)[:, :, o:],
                        in0=pt.rearrange("p (h q) -> p h q", q=QC)[:, :, o:],
                        in1=m2.rearrange("p (h q) -> p h q", q=QC)[:, :, o:])
                pt_tiles[kt] = pt

            def emit_pv_finish(dp, hh, qt, cpsum):
                h = 2 * dp + hh
                recip = work.tile([128, 1], FP32, tag="recip", name="recip")
                nc.vector.reciprocal(out=recip, in_=cpsum[:, 64:65])
                ot = outp.tile([128, 64], FP32, tag="out", name="ot")
                nc.vector.tensor_scalar_mul(ot, cpsum[:, 0:64], recip)
                nc.sync.dma_start(
                    out=out[qt * KT:(qt + 1) * KT, h * HD:(h + 1) * HD],
                    in_=ot)

            # ---- paced, demand-driven emission ----
            # Each engine executes its instructions in scheduled (= emission)
            # order, so the ScalarE exp cadence is set by how far apart
            # consecutive score matmuls sit in the PE stream.  All other PE
            # work (projections, V, PV accumulation) is split into ~0.5us
            # pieces and paced between score units with a fixed budget.
            emit_w_dma(0, 0)
            emit_w_dma(1, 0)
            nc.sync.dma_start(
                out=mask_t.rearrange("p (o q) -> p o q", q=2 * QC),
                in_=masks.rearrange("(o p) q -> p o q", p=KT))
            emit_wv_load()
            for dp in range(1, 4):
                emit_w_dma(0, dp)
                emit_w_dma(1, dp)

            from collections import deque
            MM_NS = 213          # one N=512 matmul
            PV_NS = 140          # one PV ldweights+matmul
            fillers = deque()    # (key or None, cost_ns, closure)
            emitted_keys = set()

            def push_qk(proj, dp, mc, front=False):
                key = ("qk", proj, dp, mc)
                if key in emitted_keys:
                    return
                emitted_keys.add(key)
                dst = qt_tiles[dp] if proj == 0 else kt_tiles[dp]
                psum = ps_proj.tile([128, QC], FP32, tag="proj", name="proj_ps")

                def half(first):
                    rng = range(0, 4) if first else range(4, NCT)
                    for ct in rng:
                        nc.tensor.matmul(
                            psum,
                            lhsT=wt[(proj, dp, ct)],
                            rhs=xt_tiles[ct][:, mc * QC:(mc + 1) * QC],
                            start=(ct == 0), stop=(ct == NCT - 1))
                    if not first:
                        nc.vector.tensor_copy(
                            out=dst[:, mc * QC:(mc + 1) * QC],
                            in_=psum.bitcast(F32R))
                items = [(key, 4 * MM_NS, lambda: half(True)),
                         (key, 4 * MM_NS, lambda: half(False))]
                if front:
                    fillers.extendleft(reversed(items))
                else:
                    fillers.extend(items)

            def push_v(st, front=False):
                key = ("v", st)
                if key in emitted_keys:
                    return
                emitted_keys.add(key)
                psum = ps_proj.tile([128, DG], FP32, tag="proj", name="vproj_ps")

                def half(first):
                    rng = range(0, 4) if first else range(4, NCT)
                    for ct in rng:
                        nc.tensor.matmul(
                            psum,
                            lhsT=xt_tiles[ct][:, st * KT:(st + 1) * KT],
                            rhs=wv_tiles[ct],
                            start=(ct == 0), stop=(ct == NCT - 1))
                    if not first:
                        vt = v65[st]
                        nc.vector.memset(vt, 1.0)
                        v3 = vt.rearrange("p (h e) -> p h e", e=65)
                        nc.vector.tensor_copy(
                            out=v3[:, :, 0:64],
                            in_=psum.rearrange("p (h e) -> p h e", e=64))
                items = [(key, 4 * MM_NS, lambda: half(True)),
                         (key, 4 * MM_NS, lambda: half(False))]
                if front:
                    fillers.extendleft(reversed(items))
                else:
                    fillers.extend(items)

            def push_pv(dp, hh, qt, pt_tiles):
                cpsum = ps_c.tile([128, 65], FP32, tag="c", name="c_ps")

                def batch(k0, k1):
                    for kt in range(k0, k1):
                        nc.tensor.matmul(
                            cpsum,
                            lhsT=pt_tiles[kt][:, hh * QC + (qt % 4) * KT:
                                              hh * QC + (qt % 4 + 1) * KT],
                            rhs=v65[kt][:, h65(hh)],
                            start=(kt == 0), stop=(kt == qt))
                    if k1 == qt + 1:
                        emit_pv_finish(dp, hh, qt, cpsum)
                h = 2 * dp + hh

                def h65(_hh):
                    return slice(h * 65, (h + 1) * 65)
                B = 4
                for k0 in range(0, qt + 1, B):
                    k1 = min(k0 + B, qt + 1)
                    fillers.append((None, (k1 - k0) * PV_NS,
                                    lambda k0=k0, k1=k1: batch(k0, k1)))

            def ensure(key):
                if key in emitted_keys and not any(k == key for k, c, f in fillers):
                    return
                # emit any not-yet-pushed unit, then flush its queued pieces
                if key[0] == "qk":
                    push_qk(key[1], key[2], key[3], front=True)
                else:
                    push_v(key[1], front=True)
                remaining = [(k, c, f) for k, c, f in fillers if k == key]
                for k, c, f in remaining:
                    f()
                newq = deque((k, c, f) for k, c, f in fillers if k != key)
                fillers.clear()
                fillers.extend(newq)

            def pace(budget_ns):
                spent = 0
                while fillers and spent < budget_ns:
                    k, c, f = fillers.popleft()
                    f()
                    spent += c

            # prime: first projections emitted directly (nothing to overlap)
            for key in (("qk", 0, 0, 0), ("qk", 1, 0, 0)):
                ensure(key)

            PACE_NS = 1250
            for dp in range(4):
                for qc in range(nqc):
                    n_kt = min(4 * qc + 4, nqt)
                    ensure(("qk", 0, dp, qc))
                    for m in range(qc + 1):
                        ensure(("qk", 1, dp, m))
                    # prefetch next chunk's projections into the queue front
                    if qc + 1 < nqc:
                        push_qk(0, dp, qc + 1, front=True)
                        push_qk(1, dp, qc + 1, front=True)
                    elif dp + 1 < 4:
                        push_qk(0, dp + 1, 0, front=True)
                        push_qk(1, dp + 1, 0, front=True)
                    for st in range(min(n_kt + 4, nst)):
                        push_v(st)        # queued a chunk ahead; paced
                    pt_tiles = {}
                    for kt in range(n_kt):
                        emit_score_unit(dp, qc, kt, pt_tiles)
                        if kt >= 4 * qc and kt < nqt:
                            for st in range(kt + 1):
                                ensure(("v", st))   # backstop if not drained
                            for hh in (0, 1):
                                push_pv(dp, hh, kt, pt_tiles)
                        pace(PACE_NS)
            while fillers:
                pace(10**9)

    nc.compile()
    return nc


def _causal_masks():
    """4 fp16 [128, 1024] pair tiles: mask[i, j] = (j%512 >= o*128 + i)."""
    m = np.zeros((4, KT, QC), dtype=np.float16)
    i = np.arange(KT)[:, None]
    j = np.arange(QC)[None, :]
    for o in range(4):
        m[o] = (j >= o * KT + i).astype(np.float16)
    return np.concatenate([m, m], axis=2).reshape(4 * KT, 2 * QC)


_NC_CACHE = {}


def _get_nc(seq):
    if seq not in _NC_CACHE:
        _NC_CACHE[seq] = build_nc(seq=seq)
    return _NC_CACHE[seq]


def kernel(x, Wq, Wk, Wv):
    x = np.asarray(x, dtype=np.float32)
    Wq = np.asarray(Wq, dtype=np.float32)
    Wk = np.asarray(Wk, dtype=np.float32)
    Wv = np.asarray(Wv, dtype=np.float32)
    b, seq, d = x.shape
    nc = _get_nc(seq)
    masks = _causal_masks()

    xTs = [np.ascontiguousarray(x[i].T.astype(np.float16)) for i in range(b)]
    in_maps = []
    for core in range(NCORES):
        bb, g = divmod(core, 2)
        sl = slice(g * DG, (g + 1) * DG)
        in_maps.append({
            "xT": xTs[bb],
            "wq": np.ascontiguousarray(Wq[:, sl]).astype(np.float16),
            "wk": np.ascontiguousarray(Wk[:, sl]).astype(np.float16),
            "wv": np.ascontiguousarray(Wv[:, sl]).astype(np.float16),
            "masks": masks,
        })
    res = run_bass_kernel_spmd(nc, in_maps, list(range(NCORES)))
    outp = np.empty((b, seq, d), dtype=np.float32)
    for core in range(NCORES):
        bb, g = divmod(core, 2)
        outp[bb, :, g * DG:(g + 1) * DG] = res.results[core]["out"]
    return outp
